# revision 14
# baseline (speedup 1.0000x reference)
"""Bass/Trainium2 kernel for nn_MultiHeadAttention_59459527246413.

MHA: N=4, L=2048, E=1024, H=16 heads, D=64. Returns (out, attn_weights_avg).

Sharding (8 cores): core c = (n, g) with n = c // 2 (batch), g = c % 2
(head-group of 8 heads + E/2 slice of the QKV/out projections).
No cross-device comm: host sums the two partial out-projections per batch
and averages attn weights across the two head-groups.

Per-core compute is laid out "transposed" so softmax needs no on-chip
transposes: scores are built as S_T[k_tok, q_tok] = kT.T @ qT (contraction
over head dim, K=64, two heads packed in the PE array via row tiling).
The attn_bias tile is injected into PSUM by an fp16 identity matmul before
the score matmuls accumulate on top. exp() runs on ScalarE straight out of
PSUM with the additive key-mask (+ bq·k correction) as the per-partition
activation bias. The AV matmul consumes exp scores directly (lhsT = v with
an appended ones column, so row 64 of the output accumulates the softmax
denominators for free). Normalization happens on the small outT [64, q]
tile and on the attention-average accumulator via an fp16 1/sum broadcast
tile (gpsimd cast-DMA partition-broadcast).
"""

import sys

sys.path.insert(0, "/opt/trn_rl_repo")

import numpy as np

import concourse.bass as bass
import concourse.tile as tile
from concourse import mybir
from concourse.bass_utils import run_bass_kernel_spmd

f32 = mybir.dt.float32
f16 = mybir.dt.float16
AF = mybir.ActivationFunctionType
MUL = mybir.AluOpType.mult

L = 2048  # sequence length
E = 1024  # embed dim
H8 = 8  # heads per core
D = 64  # head dim
EG = 512  # E-slice per core (H8 * D)
KT = 128  # k-token tile (partition dim)
NKT = L // KT  # 16 k tiles
QB = 512  # q-block
NQB = L // QB  # 4 q blocks
NPAIR = H8 // 2  # 4 head pairs
EXP_SHIFT = -6.0  # keep exp() outputs comfortably inside fp16 range
MASK_NEG = -30000.0  # additive key mask (exp -> 0)


def _patch_tile_drain():
    """This walrus build rejects >1 sync-wait on a CTRL instruction; Tile's
    exit drain carries one wait per DMA queue used. Split extras onto
    single-wait NOPs."""
    import concourse.tile as ctile
    from concourse.vector_clock import ScopedClock

    def _drain_and_barrier(self, tick_clock, wait_clock):
        nc = self.nc
        drain_inst = nc.sync.drain()
        wait_clock.add_sem_waits(
            drain_inst.ins, ScopedClock({None: tick_clock.global_clock})
        )
        si = drain_inst.ins.sync_info
        if si is not None and si.on_wait is not None and len(si.on_wait) > 1:
            extra = list(si.on_wait[1:])
            del si.on_wait[1:]
            for w in extra:
                nop = nc.sync.nop(nofuse=True, hint="split_drain_wait")
                nsi = nop.ins.sync_info
                if nsi is None:
                    nop.ins.sync_info = mybir.SyncInfo(on_wait=[w], on_update=[])
                else:
                    nsi.on_wait.append(w)
        nc.all_engine_barrier()
        assert self.sems is not None
        popped = nc._tile_sem_poison_stack.pop()
        assert popped is self._sem_poison
        nc.clear_and_free_semaphores(list(self.sems.allocated().values()))
        nc.all_engine_barrier()

    ctile.TileContext._drain_and_barrier = _drain_and_barrier


def _split_multi_waits(nc):
    """This walrus build allows at most one sync-wait per instruction.
    Move extra waits onto dedicated EventSemaphore instructions placed
    immediately before the owning instruction (same engine stream)."""
    import bass_rust

    counter = [0]
    for _, b in nc.bb_map.items():
        il = b.bb.instructions
        new = []
        changed = False
        for inst in il:
            si = inst.sync_info
            if si is not None and si.on_wait and len(si.on_wait) > 1:
                waits = list(si.on_wait)
                for w in waits[:-1]:
                    ev = bass_rust.InstEventSemaphore(
                        name=f"I-mwsplit-{counter[0]}", engine=inst.engine
                    )
                    counter[0] += 1
                    ev.sync_info = mybir.SyncInfo(on_wait=[w], on_update=[])
                    new.append(ev)
                inst.sync_info = mybir.SyncInfo(
                    on_wait=[waits[-1]], on_update=list(si.on_update)
                )
                changed = True
            new.append(inst)
        if changed:
            b.bb.instructions = new


def build_program() -> bass.Bass:
    _patch_tile_drain()
    nc = bass.Bass()

    queryT = nc.declare_dram_parameter("queryT", [E, L], f32, isOutput=False)
    wqT = nc.declare_dram_parameter("wqT", [E, EG], f32, isOutput=False)
    wkT = nc.declare_dram_parameter("wkT", [E, EG], f32, isOutput=False)
    wvT = nc.declare_dram_parameter("wvT", [E, EG], f32, isOutput=False)
    woT = nc.declare_dram_parameter("woT", [EG, E], f16, isOutput=False)
    bq_d = nc.declare_dram_parameter("bq", [128, H8], f32, isOutput=False)
    maskb_d = nc.declare_dram_parameter("maskb", [KT, NKT], f32, isOutput=False)
    ident_d = nc.declare_dram_parameter("ident", [128, 128], f16, isOutput=False)
    ones_d = nc.declare_dram_parameter("ones_row", [1, 128], f32, isOutput=False)
    biasT = nc.declare_dram_parameter("biasT", [H8, L, L], f16, isOutput=False)

    outT = nc.declare_dram_parameter("outT", [E, L], f32, isOutput=True)
    accO = nc.declare_dram_parameter("acc", [L, L], f16, isOutput=True)

    with tile.TileContext(nc) as tc:
        with (
            nc.allow_low_precision(reason="fp16 attn weights/acc by design"),
            tc.tile_pool(name="persist", bufs=1) as persist,
            tc.tile_pool(name="consts", bufs=1) as consts,
        ):
            # qT/kT: [512 dims, L] f32 as 4 partition tiles of [128, L]
            # tile p holds heads 2p (rows 0-63) and 2p+1 (rows 64-127)
            qT = [persist.tile([128, L], f32, tag=f"qT{i}", name=f"qT{i}") for i in range(4)]
            kT = [persist.tile([128, L], f32, tag=f"kT{i}", name=f"kT{i}") for i in range(4)]
            # v (+ones col): per k-tile [128, 8 heads * 66]; head h at cols 66h..66h+64
            v_sb = [persist.tile([128, H8 * 66], f16, tag=f"v{i}", name=f"v{i}") for i in range(NKT)]
            maskb = consts.tile([KT, NKT], f32, tag="maskb", name="maskb")
            bq_sb = consts.tile([128, H8], f32, tag="bq", name="bq_sb")
            ident = consts.tile([128, 128], f16, tag="ident", name="ident")
            ones_row = consts.tile([1, 128], f32, tag="ones_row", name="ones_row")
            # exp bias per (h, kt): col h*NKT + kt
            expb = consts.tile([KT, H8 * NKT], f32, tag="expb", name="expb")

            nc.sync.dma_start(maskb[:], maskb_d[:])
            nc.sync.dma_start(bq_sb[:], bq_d[:])
            nc.sync.dma_start(ident[:], ident_d[:])
            nc.sync.dma_start(ones_row[:], ones_d[:])

            # ---- phase 1: QKV projections ----
            with tc.tile_pool(name="qin", bufs=1) as qin:
                qchunks = []
                for a in range(8):
                    qc = qin.tile([128, L], f32, tag=f"qchunk{a}", name=f"qchunk{a}")
                    nc.sync.dma_start(qc[:], queryT[128 * a : 128 * (a + 1), :])
                    qchunks.append(qc)

                with tc.tile_pool(name="wqk", bufs=1) as wqk:
                    w_t = {}
                    for nm, dram in (("q", wqT), ("k", wkT)):
                        for a in range(8):
                            wc = wqk.tile([128, EG], f32, tag=f"w{nm}{a}", name=f"w{nm}{a}")
                            nc.sync.dma_start(wc[:], dram[128 * a : 128 * (a + 1), :])
                            w_t[nm, a] = wc
                    with tc.tile_pool(name="qkps", bufs=2, space="PSUM") as qkps:
                        for nm, dst in (("q", qT), ("k", kT)):
                            for m in range(4):
                                ps = qkps.tile([128, L], f32, tag="qkps", name="qkps_t")
                                for n in range(4):
                                    for a in range(8):
                                        nc.tensor.matmul(
                                            ps[:, QB * n : QB * (n + 1)],
                                            w_t[nm, a][:, 128 * m : 128 * (m + 1)],
                                            qchunks[a][:, QB * n : QB * (n + 1)],
                                            start=(a == 0),
                                            stop=(a == 7),
                                        )
                                nc.vector.tensor_copy(dst[m][:], ps[:])

                with tc.tile_pool(name="wv", bufs=1) as wv:
                    wv_t = []
                    for a in range(8):
                        wc = wv.tile([128, EG], f32, tag=f"wv{a}", name=f"wv{a}")
                        nc.sync.dma_start(wc[:], wvT[128 * a : 128 * (a + 1), :])
                        wv_t.append(wc)
                    with tc.tile_pool(name="vps", bufs=2, space="PSUM") as vps:
                        for m in range(NKT):
                            ps = vps.tile([128, EG], f32, tag="vps", name="vps_t")
                            for a in range(8):
                                nc.tensor.matmul(
                                    ps[:],
                                    qchunks[a][:, 128 * m : 128 * (m + 1)],
                                    wv_t[a][:],
                                    start=(a == 0),
                                    stop=(a == 7),
                                )
                            nc.gpsimd.memset(v_sb[m][:], 1.0)
                            for h in range(H8):
                                nc.scalar.copy(
                                    v_sb[m][:, 66 * h : 66 * h + 64],
                                    ps[:, 64 * h : 64 * (h + 1)],
                                )

            # ---- phase 1b: exp bias = maskb + bq.k per (h, kt) ----
            with tc.tile_pool(name="bqps", bufs=2, space="PSUM") as bqps:
                for h in range(H8):
                    ps = bqps.tile([KT, NKT], f32, tag="bqk", name="bqk_ps")
                    for kt in range(NKT):
                        nc.tensor.matmul(
                            ps[:, kt : kt + 1],
                            kT[h // 2][
                                64 * (h % 2) : 64 * (h % 2) + 64,
                                KT * kt : KT * (kt + 1),
                            ],
                            bq_sb[64 * (h % 2) : 64 * (h % 2) + 64, h : h + 1],
                            start=True,
                            stop=True,
                        )
                    nc.vector.tensor_add(
                        expb[:, h * NKT : (h + 1) * NKT], maskb[:], ps[:]
                    )

            # ---- phase 2: attention ----
            with (
                tc.tile_pool(name="wo", bufs=1) as wo_pool,
                tc.tile_pool(name="bias", bufs=3) as bias_pool,
                tc.tile_pool(name="wexp", bufs=2) as wexp_pool,
                tc.tile_pool(name="accp", bufs=1) as acc_pool,
                tc.tile_pool(name="small", bufs=2) as small_pool,
                tc.tile_pool(name="attn", bufs=1) as attn_pool,
                tc.tile_pool(name="sps", bufs=2, space="PSUM") as sps,
                tc.tile_pool(name="avps", bufs=1, space="PSUM") as avps,
                tc.tile_pool(name="ops", bufs=2, space="PSUM") as ops,
            ):
                wo_t = []
                for a in range(4):
                    wc = wo_pool.tile([128, E], f16, tag=f"wo{a}", name=f"wo{a}")
                    nc.sync.dma_start(wc[:], woT[128 * a : 128 * (a + 1), :])
                    wo_t.append(wc)

                for qb in range(NQB):
                    qs = QB * qb
                    acc_t = [
                        acc_pool.tile([128, QB], f16, tag=f"acc{kt}", name=f"acc{kt}")
                        for kt in range(NKT)
                    ]
                    attn_p = [
                        attn_pool.tile([128, QB], f16, tag=f"ap{a}", name=f"ap{a}") for a in range(4)
                    ]

                    for pair in range(NPAIR):
                        hA, hB = 2 * pair, 2 * pair + 1
                        w_pair = wexp_pool.tile([128, NKT * 1024], f16, tag="wpair", name="w_pair")
                        ov = avps.tile([65, 1024], f32, tag="ovps", name="ov")

                        for kt in range(NKT):
                            s_ps = sps.tile([128, 1024], f32, tag="sps", name="s_ps")
                            b_sb = bias_pool.tile([128, 1024], f16, tag="btile", name="b_sb")
                            nc.sync.dma_start(
                                b_sb[:, 0:512],
                                biasT[hA, KT * kt : KT * (kt + 1), qs : qs + QB],
                            )
                            nc.sync.dma_start(
                                b_sb[:, 512:1024],
                                biasT[hB, KT * kt : KT * (kt + 1), qs : qs + QB],
                            )
                            for hh in (0, 1):
                                c0 = 512 * hh
                                # bias inject (fp16 identity matmul), then scores
                                nc.tensor.matmul(
                                    s_ps[:, c0 : c0 + 512],
                                    ident[:],
                                    b_sb[:, c0 : c0 + 512],
                                    start=True,
                                    stop=False,
                                )
                                nc.tensor.matmul(
                                    s_ps[:, c0 : c0 + 512],
                                    kT[pair][
                                        64 * hh : 64 * hh + 64, KT * kt : KT * (kt + 1)
                                    ],
                                    qT[pair][64 * hh : 64 * hh + 64, qs : qs + QB],
                                    start=False,
                                    stop=True,
                                )
                            # W = exp(S + biasT + mask + bq.k + shift) -> fp16
                            wt = w_pair[:, 1024 * kt : 1024 * (kt + 1)]
                            for hh, h in ((0, hA), (1, hB)):
                                nc.scalar.activation(
                                    wt[:, 512 * hh : 512 * hh + 512],
                                    s_ps[:, 512 * hh : 512 * hh + 512],
                                    AF.Exp,
                                    bias=expb[:, h * NKT + kt : h * NKT + kt + 1],
                                    scale=1.0,
                                )
                            # AV: outT[d(+ones), q] += v_aug.T @ W
                            for hh, h in ((0, hA), (1, hB)):
                                nc.tensor.matmul(
                                    ov[:, 512 * hh : 512 * hh + 512],
                                    v_sb[kt][:, 66 * h : 66 * h + 65],
                                    wt[:, 512 * hh : 512 * hh + 512],
                                    start=(kt == 0),
                                    stop=(kt == NKT - 1),
                                )

                        for hh, h in ((0, hA), (1, hB)):
                            c0 = 512 * hh
                            rinv = small_pool.tile([1, QB], f32, tag="rinv", name="rinv")
                            nc.vector.reciprocal(rinv[:], ov[64:65, c0 : c0 + 512])
                            rb_ps = ops.tile([128, QB], f32, tag="ops", name="rb_ps")
                            nc.tensor.matmul(
                                rb_ps[:], ones_row[:], rinv[:], start=True, stop=True
                            )
                            rb = small_pool.tile([128, QB], f16, tag=f"rb{hh}", name=f"rb{hh}")
                            nc.scalar.copy(rb[:], rb_ps[:])
                            # attn_partT rows for this head = outT * r
                            nc.vector.tensor_mul(
                                attn_p[h // 2][64 * (h % 2) : 64 * (h % 2) + 64, :],
                                ov[0:64, c0 : c0 + 512],
                                rb[0:64, :],
                            )
                            # acc += W * r (pair 0 writes directly; DVE/gpsimd split)
                            for kt in range(NKT):
                                wt = w_pair[:, 1024 * kt + c0 : 1024 * kt + c0 + 512]
                                eng = nc.gpsimd if kt % 3 == 2 else nc.vector
                                if pair == 0 and hh == 0:
                                    eng.tensor_mul(acc_t[kt][:], wt[:], rb[:])
                                else:
                                    tmp = small_pool.tile([128, QB], f16, tag=f"tmp{kt % 4}", name=f"tmp{kt % 4}")
                                    eng.tensor_mul(tmp[:], wt[:], rb[:])
                                    eng.tensor_add(acc_t[kt][:], acc_t[kt][:], tmp[:])

                    # out projection for this q block
                    for m in range(8):
                        ps = ops.tile([128, QB], f32, tag="ops", name="ops_t")
                        for a in range(4):
                            nc.tensor.matmul(
                                ps[:],
                                wo_t[a][:, 128 * m : 128 * (m + 1)],
                                attn_p[a][:],
                                start=(a == 0),
                                stop=(a == 3),
                            )
                        o_sb = small_pool.tile([128, QB], f32, tag="osb", name="o_sb")
                        nc.scalar.copy(o_sb[:], ps[:])
                        nc.sync.dma_start(
                            outT[128 * m : 128 * (m + 1), qs : qs + QB], o_sb[:]
                        )

                    for kt in range(NKT):
                        nc.sync.dma_start(
                            accO[KT * kt : KT * (kt + 1), qs : qs + QB], acc_t[kt][:]
                        )

    _split_multi_waits(nc)
    return nc


_PROGRAM = None


def _get_program():
    global _PROGRAM
    if _PROGRAM is None:
        _PROGRAM = build_program()
    return _PROGRAM


def _build_in_maps(query, attn_bias, mask, Wq, bq, Wk, Wv, Wo):
    scale = 1.0 / np.sqrt(D)
    ident = np.eye(128, dtype=np.float16)

    biasT_g, wq_g, wk_g, wv_g, wo_g, bq_g = [], [], [], [], [], []
    for g in range(2):
        sl = slice(EG * g, EG * (g + 1))
        biasT_g.append(
            np.ascontiguousarray(
                attn_bias[0, H8 * g : H8 * (g + 1)].transpose(0, 2, 1)
            ).astype(np.float16)
        )
        wq_g.append(np.ascontiguousarray((Wq[sl, :] * scale).T))
        wk_g.append(np.ascontiguousarray(Wk[sl, :].T))
        wv_g.append(np.ascontiguousarray(Wv[sl, :].T))
        wo_g.append(np.ascontiguousarray(Wo[:, sl].T).astype(np.float16))
        bqa = np.zeros((128, H8), np.float32)
        for h in range(H8):
            bqa[64 * (h % 2) : 64 * (h % 2) + 64, h] = bq[sl][h * D : (h + 1) * D] * scale
        bq_g.append(bqa)

    in_maps = []
    for c in range(8):
        n, g = divmod(c, 2)
        madd = np.where(mask[n], MASK_NEG, 0.0).astype(np.float32) + EXP_SHIFT
        in_maps.append(
            {
                "queryT": np.ascontiguousarray(query[n].T),
                "wqT": wq_g[g],
                "wkT": wk_g[g],
                "wvT": wv_g[g],
                "woT": wo_g[g],
                "bq": bq_g[g],
                "maskb": np.ascontiguousarray(madd.reshape(NKT, KT).T),
                "ident": ident,
                "ones_row": np.ones((1, 128), np.float32),
                "biasT": biasT_g[g],
            }
        )
    return in_maps


def run_traced(np_inputs, trace_cores=None):
    in_maps = _build_in_maps(
        np.asarray(np_inputs["query"], np.float32),
        np.asarray(np_inputs["attn_bias"], np.float32),
        np.asarray(np_inputs["mask"]),
        np.asarray(np_inputs["Wq"], np.float32),
        np.asarray(np_inputs["bq"], np.float32),
        np.asarray(np_inputs["Wk"], np.float32),
        np.asarray(np_inputs["Wv"], np.float32),
        np.asarray(np_inputs["Wo"], np.float32),
    )
    return run_bass_kernel_spmd(
        _get_program(),
        in_maps,
        list(range(8)),
        trace=True,
        tmpdir="/root/problem/traces",
        trace_cores=trace_cores,
    )


def kernel(query, attn_bias, mask, Wq, bq, Wk, bk, Wv, bv, Wo, bo, _want_results=False):
    query = np.asarray(query, np.float32)
    attn_bias = np.asarray(attn_bias, np.float32)
    mask = np.asarray(mask)
    Wq = np.asarray(Wq, np.float32)
    bq = np.asarray(bq, np.float32)
    Wk = np.asarray(Wk, np.float32)
    Wv = np.asarray(Wv, np.float32)
    bv = np.asarray(bv, np.float32)
    Wo = np.asarray(Wo, np.float32)
    bo = np.asarray(bo, np.float32)

    N = query.shape[0]
    in_maps = _build_in_maps(query, attn_bias, mask, Wq, bq, Wk, Wv, Wo)
    res = run_bass_kernel_spmd(_get_program(), in_maps, list(range(8)))

    bo_eff = bo + bv @ Wo.T
    out = np.empty((N, L, E), np.float32)
    avg = np.empty((N, L, L), np.float32)
    for n in range(N):
        r0, r1 = res.results[2 * n], res.results[2 * n + 1]
        out[n] = (r0["outT"] + r1["outT"]).T + bo_eff
        avg[n] = (r0["acc"].astype(np.float32) + r1["acc"].astype(np.float32)).T / 16.0
    if _want_results:
        return (out, avg), res
    return out, avg
